# revision 5
# baseline (speedup 1.0000x reference)
"""Bilateral LSTM cell on 8 Trainium2 NeuronCores.

Sharding: data-parallel over the batch dim (4096 -> 512 rows/core); the 12
weight matrices are replicated, concatenated on host into one (2560, 4096)
matrix whose columns are permuted into 4 "quarter" blocks of
[i|f|o|c] x 256 h-columns each, so all four gate slices for a given range of
h-columns live in adjacent PSUM banks and the elementwise LSTM math is local.

Per core:
  gates = [x | h_prev | h_cross] @ Wall        (512 x 2560) @ (2560 x 4096)
  via fp32r matmuls: stationary = transposed activation 128x128 blocks
  (PE-transposed on device), moving = weight tiles [128, 512].
  PSUM: 4 m-tiles x 2 half-quarter banks accumulate over 20 k-tiles.
  ScalarE applies sigmoid/tanh out of PSUM; VectorE does the c/h updates.
"""

from contextlib import ExitStack

import numpy as np

import concourse.bass as bass
import concourse.mybir as mybir
from concourse import bacc
import concourse.tile as tile
from concourse import bass_utils
from concourse.masks import make_identity

B, D_IN, H = 4096, 512, 1024
N_CORES = 8
BS = B // N_CORES            # 512 batch rows per core
MT = BS // 128               # 4 m-tiles (batch blocks)
KD = D_IN // 128             # 4 k-tiles from x
KH = H // 128                # 8 k-tiles from h_prev / h_cross
KTOT = KD + 2 * KH           # 20 contraction tiles
NQ = 4                       # gate-column quarters
QC = H // NQ                 # 256 h-cols per quarter
F32 = mybir.dt.float32
F32R = mybir.dt.float32r
SIG = mybir.ActivationFunctionType.Sigmoid
TANH = mybir.ActivationFunctionType.Tanh


def _kernel_body(ctx, tc, x, hp, cp, hx, wall, state):
    nc = tc.nc

    singles = ctx.enter_context(tc.tile_pool(name="singles", bufs=1))
    stage = ctx.enter_context(tc.tile_pool(name="stage", bufs=3))
    persist = ctx.enter_context(tc.tile_pool(name="persist", bufs=1))
    wpool = ctx.enter_context(tc.tile_pool(name="wpool", bufs=4))
    pspool = ctx.enter_context(tc.tile_pool(name="pspool", bufs=8, space="PSUM"))
    gpool = ctx.enter_context(tc.tile_pool(name="gpool", bufs=3))
    epool = ctx.enter_context(tc.tile_pool(name="epool", bufs=3))
    opool = ctx.enter_context(tc.tile_pool(name="opool", bufs=4))

    identity = singles.tile([128, 128], F32)
    make_identity(nc, identity)

    # Transposed activations, [128 feature-partitions, k-block, batch]:
    # xT k-blocks 0..3, hpT 0..7, hxT 0..7 -> 20 stationary k-tiles total.
    xT = persist.tile([128, KD, BS], F32R, tag="xT")
    hpT = persist.tile([128, KH, BS], F32R, tag="hpT")
    hxT = persist.tile([128, KH, BS], F32R, tag="hxT")

    # Load natural-layout activations and transpose 128x128 blocks on PE.
    for m in range(MT):
        for src, dstT, nk in ((x, xT, KD), (hp, hpT, KH), (hx, hxT, KH)):
            a_tile = stage.tile([128, nk * 128], F32, tag="astage")
            nc.sync.dma_start(out=a_tile, in_=src[m * 128:(m + 1) * 128, :])
            for g in range(nk // 4):  # groups of 4 k-blocks -> one PSUM bank
                pt = pspool.tile([128, 512], F32, tag="ps")
                for kb in range(4):
                    nc.tensor.transpose(
                        pt[:, kb * 128:(kb + 1) * 128],
                        a_tile[:, (g * 4 + kb) * 128:(g * 4 + kb + 1) * 128],
                        identity,
                    )
                nc.vector.tensor_copy(
                    out=dstT[:, g * 4:(g + 1) * 4, m * 128:(m + 1) * 128],
                    in_=pt.rearrange("p (k b) -> p k b", k=4),
                )

    def actT_slice(k, m):
        if k < KD:
            t, kk = xT, k
        elif k < KD + KH:
            t, kk = hpT, k - KD
        else:
            t, kk = hxT, k - KD - KH
        return t[:, kk, m * 128:(m + 1) * 128]

    for q in range(NQ):
        qcol = q * 4 * QC  # 1024 gate-cols per quarter: [i|f|o|c] x 256
        accs = [
            [pspool.tile([128, 512], F32, tag="ps", name=f"acc_q{q}_m{_m}_{_j}")
             for _j in range(2)]
            for _m in range(MT)
        ]
        for k in range(KTOT):
            wt = wpool.tile([128, 4 * QC], F32R, tag="wt")
            nc.sync.dma_start(
                out=wt, in_=wall[k * 128:(k + 1) * 128, qcol:qcol + 4 * QC]
            )
            wtr = wt
            for m in range(MT):
                lhsT = actT_slice(k, m)
                for j in range(2):
                    nc.tensor.matmul(
                        accs[m][j],
                        lhsT,
                        wtr[:, j * 512:(j + 1) * 512],
                        start=(k == 0),
                        stop=(k == KTOT - 1),
                    )
        for m in range(MT):
            rows = slice(m * 128, (m + 1) * 128)
            # accs[m][0] = [i(256) | f(256)], accs[m][1] = [o(256) | c_(256)]
            sif = gpool.tile([128, 512], F32, tag="sif")
            nc.scalar.activation(sif, accs[m][0], SIG)
            so = gpool.tile([128, QC], F32, tag="so")
            nc.scalar.activation(so, accs[m][1][:, 0:QC], SIG)
            ct = gpool.tile([128, QC], F32, tag="ct")
            nc.scalar.activation(ct, accs[m][1][:, QC:2 * QC], TANH)

            cprev = epool.tile([128, QC], F32, tag="cprev")
            nc.sync.dma_start(out=cprev, in_=cp[rows, q * QC:(q + 1) * QC])

            t1 = epool.tile([128, QC], F32, tag="t1")
            nc.vector.tensor_mul(t1, sif[:, 0:QC], ct)        # i * c~
            t2 = epool.tile([128, QC], F32, tag="t2")
            nc.vector.tensor_mul(t2, sif[:, QC:2 * QC], cprev)  # f * c_prev
            cnew = opool.tile([128, QC], F32, tag="cnew")
            nc.vector.tensor_add(cnew, t1, t2)

            tch = epool.tile([128, QC], F32, tag="tch")
            nc.scalar.activation(tch, cnew, TANH)
            hnew = opool.tile([128, QC], F32, tag="hnew")
            nc.vector.tensor_mul(hnew, so, tch)

            nc.sync.dma_start(
                out=state[rows, q * QC:(q + 1) * QC], in_=hnew
            )
            nc.sync.dma_start(
                out=state[rows, H + q * QC:H + (q + 1) * QC], in_=cnew
            )


def build_nc(iters: int = 1):
    nc = bacc.Bacc(
        "TRN2",
        target_bir_lowering=False,
        debug=False,
        enable_asserts=False,
        num_devices=N_CORES,
    )
    x = nc.dram_tensor("x", (BS, D_IN), F32, kind="ExternalInput").ap()
    hp = nc.dram_tensor("hp", (BS, H), F32, kind="ExternalInput").ap()
    cp = nc.dram_tensor("cp", (BS, H), F32, kind="ExternalInput").ap()
    hx = nc.dram_tensor("hx", (BS, H), F32, kind="ExternalInput").ap()
    wall = nc.dram_tensor(
        "wall", (D_IN + 2 * H, 4 * H), F32R, kind="ExternalInput"
    ).ap()
    state = nc.dram_tensor("state", (BS, 2 * H), F32, kind="ExternalOutput").ap()

    with tile.TileContext(nc) as tc, ExitStack() as ctx:
        if iters == 1:
            _kernel_body(ctx, tc, x, hp, cp, hx, wall, state)
        else:
            import concourse.mybir as _mb
            with tc.For_i(0, iters, 1,
                          hint_engines=(_mb.EngineType.PE,
                                        _mb.EngineType.DVE,
                                        _mb.EngineType.Activation,
                                        _mb.EngineType.SP)):
                with ExitStack() as inner:
                    _kernel_body(inner, tc, x, hp, cp, hx, wall, state)
    nc.compile()
    return nc


def build_wall(Wi, Ui, Vi, Wf, Uf, Vf, Wo, Uo, Vo, Wc, Uc, Vc):
    """(2560, 4096) weight matrix; columns permuted into quarter blocks of
    [i|f|o|c] x 256 h-cols."""
    wall = np.empty((D_IN + 2 * H, 4 * H), np.float32)
    for q in range(NQ):
        s = slice(q * QC, (q + 1) * QC)
        base = q * 4 * QC
        for gi, (wg, ug, vg) in enumerate(
            ((Wi, Ui, Vi), (Wf, Uf, Vf), (Wo, Uo, Vo), (Wc, Uc, Vc))
        ):
            c = slice(base + gi * QC, base + (gi + 1) * QC)
            wall[0:D_IN, c] = wg[:, s]
            wall[D_IN:D_IN + H, c] = ug[:, s]
            wall[D_IN + H:, c] = vg[:, s]
    return wall


_NC_CACHE = []


def kernel(x, hidden_memory_tm1, hidden_memory_tm2,
           Wi, Ui, Vi, Wf, Uf, Vf, Wo, Uo, Vo, Wc, Uc, Vc):
    if not _NC_CACHE:
        _NC_CACHE.append(build_nc())
    nc = _NC_CACHE[0]

    wall = build_wall(Wi, Ui, Vi, Wf, Uf, Vf, Wo, Uo, Vo, Wc, Uc, Vc)
    x = np.asarray(x, np.float32)
    tm1 = np.asarray(hidden_memory_tm1, np.float32)
    tm2 = np.asarray(hidden_memory_tm2, np.float32)

    in_maps = []
    for c in range(N_CORES):
        rs = slice(c * BS, (c + 1) * BS)
        in_maps.append({
            "x": np.ascontiguousarray(x[rs]),
            "hp": np.ascontiguousarray(tm1[rs, :H]),
            "cp": np.ascontiguousarray(tm1[rs, H:]),
            "hx": np.ascontiguousarray(tm2[rs, :H]),
            "wall": wall,
        })

    res = bass_utils.run_bass_kernel_spmd(nc, in_maps, core_ids=list(range(N_CORES)))
    state = np.concatenate([r["state"] for r in res.results], axis=0)
    h = np.ascontiguousarray(state[:, :H])
    return h, state
